# revision 16
# baseline (speedup 1.0000x reference)
"""Multi-head self-attention TRN2 Bass kernel.

Problem: x[2, 2048, 1024], 16 heads x 64 dim, fp32.
Sharding: 8 cores = 2 batches x 4 head-groups (4 heads each).
Each core computes its batch's partial output (its 4 heads through
QKV -> attention -> output projection rows); host sums the 4 partials
per batch and adds bo.

Per-core structure (avoids every attention transpose):
  - x^T loaded straight from HBM via DMA xbar transpose (x cast to bf16
    on host).
  - q^T, k^T [256, 2048] bf16  (head h at partitions (h%2)*64 of tile h//2)
  - V' [2048, 4, 65] bf16  (per head: V columns + a ones column)
  - scores computed TRANSPOSED: S^T[k,q] = k^T.T @ q^T as row-tiled
    head-PAIRS (two concurrent K=64 matmuls); 1/sqrt(hd) folded into
    Wq/bq on host.
  - exp on ACT -> A^T bf16, directly the moving operand of
    out^T[65, q] = V'^T @ A^T; row 64 = softmax row sums (ones trick).
  - normalize on DVE (fast reciprocal + gpsimd partition_broadcast).
  - out_proj: head pairs stacked to K=128, software-pipelined one
    pair-block later so its inputs are always ready.
  - the exp stream is the bottleneck; q-chunk 0's scores/exp start right
    after kT + qT chunk 0, with the rest of phase A (qT chunks 1-3, V)
    emitted as PE filler inside that exp-paced loop. Dummy matmuls fill
    the remaining PE bubbles to keep the HAM clock-gate at 2.4 GHz.
"""

import numpy as np

S = 2048          # sequence length per batch
H = 1024          # hidden
G = 256           # head-group width (4 heads x 64)
HD = 65           # V' columns per head (64 + ones)
NHL = 4           # heads per core
N_CORES = 8

_CACHE = {}


def _build():
    if "nc" in _CACHE:
        return _CACHE["nc"]

    import concourse.bass as bass
    import concourse.mybir as mybir
    import concourse.tile as tile
    from concourse import bacc

    f32 = mybir.dt.float32
    bf16 = mybir.dt.bfloat16
    EXP = mybir.ActivationFunctionType.Exp

    nc = bacc.Bacc("TRN2", target_bir_lowering=False, debug=False,
                   num_devices=N_CORES)

    x_in = nc.dram_tensor("x", [S, H], bf16, kind="ExternalInput")
    wq_in = nc.dram_tensor("wq", [H, G], bf16, kind="ExternalInput")
    wk_in = nc.dram_tensor("wk", [H, G], bf16, kind="ExternalInput")
    wv_in = nc.dram_tensor("wv", [H, G], bf16, kind="ExternalInput")
    bq_in = nc.dram_tensor("bq", [G, 1], f32, kind="ExternalInput")
    bk_in = nc.dram_tensor("bk", [G, 1], f32, kind="ExternalInput")
    bv_in = nc.dram_tensor("bv", [G], f32, kind="ExternalInput")
    wo_in = nc.dram_tensor("wo", [NHL, 64, H], bf16, kind="ExternalInput")
    out_d = nc.dram_tensor("out", [S, H], f32, kind="ExternalOutput")

    with tile.TileContext(nc) as tc:
        with (
            tc.tile_pool(name="persist", bufs=1) as persist,
            tc.tile_pool(name="atq0", bufs=1) as atq0_pool,
        ):
            qT = persist.tile([128, 2, S], bf16)     # [qd, m, s]
            kT = persist.tile([128, 2, S], bf16)
            vp = persist.tile([128, 16, NHL, HD], bf16)  # [s-part, st, h, col]
            bq_sb = persist.tile([128, 2, 1], f32)
            bk_sb = persist.tile([128, 2, 1], f32)
            bv_bc = persist.tile([128, G], f32)
            wo_pr = persist.tile([128, 2, H], bf16)

            nc.sync.dma_start(
                out=bq_sb, in_=bq_in.ap().rearrange("(m p) o -> p m o", p=128))
            nc.sync.dma_start(
                out=bk_sb, in_=bk_in.ap().rearrange("(m p) o -> p m o", p=128))
            # broadcast bv along partitions (stride-0 partition AP)
            bv_ap = bass.AP(tensor=bv_in, offset=0, ap=[[0, 128], [1, G]])
            nc.gpsimd.dma_start(out=bv_bc, in_=bv_ap)
            # Wo as stacked head pairs: [two*64+p, pr, n]
            nc.sync.dma_start(
                out=wo_pr,
                in_=wo_in.ap().rearrange("(pr two) p n -> (two p) pr n", two=2))

            # ones columns of V'
            nc.gpsimd.memset(vp[:, :, :, 64:65], 1.0)

            # q-chunk 0 attention weights, exp'd during phase A
            atq0 = [atq0_pool.tile([128, 2, 16, 512], bf16, name=f"atq0_{mt}")
                    for mt in range(2)]

            # ------- Phase A + q-chunk-0 scores/exp overlap -------
            with (
                tc.tile_pool(name="wqkv", bufs=1) as w_pool,
                tc.tile_pool(name="xT", bufs=1) as xT_pool,
                tc.tile_pool(name="ps_a", bufs=2, space="PSUM") as ps_a,
                tc.tile_pool(name="ps_se", bufs=2, space="PSUM") as ps_se,
            ):
                wq_sb = w_pool.tile([128, 8, G], bf16)
                wk_sb = w_pool.tile([128, 8, G], bf16)
                wv_sb = w_pool.tile([128, 8, G], bf16)
                nc.sync.dma_start(
                    out=wq_sb, in_=wq_in.ap().rearrange("(t p) d -> p t d", p=128))
                nc.sync.dma_start(
                    out=wk_sb, in_=wk_in.ap().rearrange("(t p) d -> p t d", p=128))
                nc.sync.dma_start(
                    out=wv_sb, in_=wv_in.ap().rearrange("(t p) d -> p t d", p=128))

                xT = xT_pool.tile([128, 8, S], bf16)   # [h-part, ht, s]
                # chunk-wise xbar-transpose loads so kT can start early
                for jc in range(4):
                    for ht in range(8):
                        nc.sync.dma_start(
                            out=xT[:, ht, jc * 512:(jc + 1) * 512],
                            in_=x_in.ap()[jc * 512:(jc + 1) * 512,
                                          ht * 128:(ht + 1) * 128],
                            transpose=True)

                def dummy_a(n=512):
                    ps_d = ps_a.tile([128, 512], f32, tag="dum", bufs=1)
                    nc.tensor.matmul(
                        ps_d[:, 0:n], lhsT=wq_sb[:, 0, 0:128],
                        rhs=wq_sb[:, 0:2, :].rearrange("p a b -> p (a b)")[:, 0:n],
                        start=True, stop=True)

                def qk_half(w_sb, b_sb, dst, jc, m, half, st):
                    sl = slice(jc * 512, (jc + 1) * 512)
                    if half == 0:
                        st["ps"] = ps_a.tile([128, 512], f32, tag="qk",
                                             name=f"psq_{id(w_sb)}_{jc}_{m}")
                    for ht in range(half * 4, half * 4 + 4):
                        nc.tensor.matmul(
                            st["ps"],
                            lhsT=w_sb[:, ht, m * 128:(m + 1) * 128],
                            rhs=xT[:, ht, sl],
                            start=(ht == 0), stop=(ht == 7))
                    if half == 1:
                        nc.vector.tensor_scalar_add(
                            dst[:, m, sl], st["ps"], b_sb[:, m, :])

                def v_unit(st16):
                    ps_vt = ps_a.tile([128, 512], f32, tag="qk",
                                      name=f"psv_{st16}")
                    for ht in range(8):
                        nc.tensor.matmul(
                            ps_vt[:, 0:G],
                            lhsT=xT[:, ht, st16 * 128:(st16 + 1) * 128],
                            rhs=wv_sb[:, ht, :],
                            start=(ht == 0), stop=(ht == 7))
                    nc.vector.tensor_add(
                        vp[:, st16, :, 0:64],
                        ps_vt[:, 0:G].rearrange("p (h d) -> p h d", h=NHL),
                        bv_bc.rearrange("p (h d) -> p h d", h=NHL))

                # k^T (all chunks) and q^T chunk 0 first: unblocks scores
                for jc in range(4):
                    for m in range(2):
                        st = {}
                        qk_half(wk_sb, bk_sb, kT, jc, m, 0, st)
                        qk_half(wk_sb, bk_sb, kT, jc, m, 1, st)
                        dummy_a()
                        dummy_a()
                for m in range(2):
                    st = {}
                    qk_half(wq_sb, bq_sb, qT, 0, m, 0, st)
                    qk_half(wq_sb, bq_sb, qT, 0, m, 1, st)

                # filler units: rest of phase A, emitted inside the
                # exp-paced q-chunk-0 scores loop
                filler = []
                for jc in range(1, 4):
                    for m in range(2):
                        st = {}
                        filler.append(
                            lambda jc=jc, m=m, st=st: qk_half(
                                wq_sb, bq_sb, qT, jc, m, 0, st))
                        filler.append(
                            lambda jc=jc, m=m, st=st: qk_half(
                                wq_sb, bq_sb, qT, jc, m, 1, st))
                for st16 in range(16):
                    filler.append(lambda st16=st16: v_unit(st16))

                # q-chunk 0 scores + exp, phase A as PE filler
                for mt in range(2):
                    for kt in range(16):
                        ps_s = ps_se.tile([128, 2, 512], f32, tag="se")
                        for hh in range(2):
                            nc.tensor.matmul(
                                ps_s[:, hh, :],
                                lhsT=kT[hh * 64:hh * 64 + 64, mt,
                                        kt * 128:(kt + 1) * 128],
                                rhs=qT[hh * 64:hh * 64 + 64, mt, 0:512],
                                start=True, stop=True)
                        nc.scalar.activation(
                            out=atq0[mt][:, :, kt, :], in_=ps_s, func=EXP)
                        if filler:
                            filler.pop(0)()
                        else:
                            dummy_a()
                while filler:
                    filler.pop(0)()

            # ---------------- Phase B: attention + out_proj ----------------
            with (
                tc.tile_pool(name="at_roll", bufs=2) as at_pool,
                tc.tile_pool(name="outP", bufs=4) as op_pool,
                tc.tile_pool(name="tmpo", bufs=1) as tmpo_pool,
                tc.tile_pool(name="sums", bufs=4) as sums_pool,
                tc.tile_pool(name="rbc", bufs=3) as rbc_pool,
                tc.tile_pool(name="osb", bufs=2) as osb_pool,
                tc.tile_pool(name="ps_s", bufs=2, space="PSUM") as ps_s_pool,
                tc.tile_pool(name="ps_av", bufs=2, space="PSUM") as ps_av_pool,
                tc.tile_pool(name="ps_op", bufs=1, space="PSUM") as ps_op_pool,
            ):
                def dummy(n):
                    ps_d = ps_op_pool.tile([128, 512], f32, tag="dummy")
                    nc.tensor.matmul(ps_d[:, 0:n], lhsT=kT[:, 0, 0:128],
                                     rhs=qT[:, 0, 0:n], start=True, stop=True)

                def norm_head(outP, ps_av, hh, qc, mt):
                    # evacuate PSUM right away to release the bank; run the
                    # normalize chain from SBUF
                    uout = tmpo_pool.tile([HD, 512], f32, tag="uout",
                                          name=f"uo_{qc}_{mt}_{hh}", bufs=4)
                    nc.vector.tensor_copy(uout, ps_av)
                    sums = sums_pool.tile([1, 512], f32, tag="sums",
                                          name=f"sm_{qc}_{mt}_{hh}")
                    nc.vector.tensor_copy(sums, uout[64:65, :])
                    recip = sums_pool.tile([1, 512], f32, tag="recip",
                                           name=f"rc_{qc}_{mt}_{hh}")
                    nc.vector.reciprocal_approx_fast(out=recip, in_=sums)
                    rbc = rbc_pool.tile([64, 512], f32, tag="rbc",
                                        name=f"rb_{qc}_{mt}_{hh}")
                    nc.gpsimd.partition_broadcast(rbc, recip)
                    nc.vector.tensor_mul(
                        outP[hh * 64:hh * 64 + 64, :], uout[0:64, :], rbc)

                def emit_oproj(qc, outPs):
                    # out_proj for q-chunk qc (K=128 stacked pairs)
                    for qt in range(4):
                        osb = osb_pool.tile([128, H], f32, tag="osb",
                                            name=f"osb_{qc}_{qt}")
                        for ncx in range(2):
                            ps_op = ps_op_pool.tile(
                                [128, 512], f32, tag="oproj",
                                name=f"pso_{qc}_{qt}_{ncx}")
                            for pr in range(2):
                                nc.tensor.matmul(
                                    ps_op,
                                    lhsT=outPs[pr][:, qt * 128:(qt + 1) * 128],
                                    rhs=wo_pr[:, pr, ncx * 512:(ncx + 1) * 512],
                                    start=(pr == 0), stop=(pr == 1))
                            nc.vector.tensor_copy(
                                osb[:, ncx * 512:(ncx + 1) * 512], ps_op)
                        nc.sync.dma_start(
                            out=out_d.ap()[qc * 512 + qt * 128:
                                           qc * 512 + (qt + 1) * 128, :],
                            in_=osb)

                # q-chunk 0: attn@V from the pre-exp'd atq0 tiles
                outPs0 = []
                for mt in range(2):
                    ps_avs = [ps_av_pool.tile([HD, 512], f32, tag="av",
                                              name=f"av0_{mt}_{hh}")
                              for hh in range(2)]
                    for kt in range(16):
                        for hh in range(2):
                            nc.tensor.matmul(
                                ps_avs[hh],
                                lhsT=vp[:, kt, 2 * mt + hh, :],
                                rhs=atq0[mt][:, hh, kt, :],
                                start=(kt == 0), stop=(kt == 15))
                    outP = op_pool.tile([128, 512], bf16, tag="outP",
                                        name=f"outP_0_{mt}")
                    for hh in range(2):
                        norm_head(outP, ps_avs[hh], hh, 0, mt)
                    outPs0.append(outP)
                    for _ in range(2):
                        dummy(512)
                prev = (0, outPs0)

                for qc in range(1, 4):  # q-chunks 1..3
                    qsl = slice(qc * 512, (qc + 1) * 512)
                    outPs = []
                    for mt in range(2):  # head pair (2mt, 2mt+1)
                        attnT = at_pool.tile([128, 2, 4, 512], bf16,
                                             tag="at", name=f"at_{qc}_{mt}")
                        ps_avs = [ps_av_pool.tile([HD, 512], f32, tag="av",
                                                  name=f"av_{qc}_{mt}_{hh}")
                                  for hh in range(2)]
                        for kt in range(16):
                            ps_s = ps_s_pool.tile([128, 2, 512], f32, tag="s")
                            for hh in range(2):
                                nc.tensor.matmul(
                                    ps_s[:, hh, :],
                                    lhsT=kT[hh * 64:hh * 64 + 64, mt,
                                            kt * 128:(kt + 1) * 128],
                                    rhs=qT[hh * 64:hh * 64 + 64, mt, qsl],
                                    start=True, stop=True)
                            nc.scalar.activation(
                                out=attnT[:, :, kt % 4, :], in_=ps_s, func=EXP)
                            for hh in range(2):
                                nc.tensor.matmul(
                                    ps_avs[hh],
                                    lhsT=vp[:, kt, 2 * mt + hh, :],
                                    rhs=attnT[:, hh, kt % 4, :],
                                    start=(kt == 0), stop=(kt == 15))
                            dummy(256)
                        outP = op_pool.tile([128, 512], bf16, tag="outP",
                                            name=f"outP_{qc}_{mt}")
                        for hh in range(2):
                            norm_head(outP, ps_avs[hh], hh, qc, mt)
                        outPs.append(outP)
                        for _ in range(2):
                            dummy(512)
                        if prev is not None and mt == 0:
                            # out_proj of the PREVIOUS q-chunk, pipelined in
                            # here so its normalize chains are long finished
                            emit_oproj(*prev)
                            prev = None
                    prev = (qc, outPs)
                # cover the last normalize chain, then final out_proj
                for _ in range(10):
                    dummy(512)
                emit_oproj(*prev)

    nc.compile()
    _CACHE["nc"] = nc
    return nc


def make_in_maps(x, Wq, bq, Wk, bk, Wv, bv, Wo):
    import ml_dtypes
    bf = ml_dtypes.bfloat16

    x = np.asarray(x, dtype=np.float32)
    Wq = np.asarray(Wq, dtype=np.float32)
    bq = np.asarray(bq, dtype=np.float32)
    Wk = np.asarray(Wk, dtype=np.float32)
    bk = np.asarray(bk, dtype=np.float32)
    Wv = np.asarray(Wv, dtype=np.float32)
    bv = np.asarray(bv, dtype=np.float32)
    Wo = np.asarray(Wo, dtype=np.float32)

    scale = np.float32(1.0 / 8.0)  # 1/sqrt(64)

    in_maps = []
    for core in range(N_CORES):
        b = core // 4
        g = core % 4
        cs = slice(g * G, (g + 1) * G)
        in_maps.append({
            "x": np.ascontiguousarray(x[b]).astype(bf),
            "wq": np.ascontiguousarray(Wq[:, cs] * scale).astype(bf),
            "wk": np.ascontiguousarray(Wk[:, cs]).astype(bf),
            "wv": np.ascontiguousarray(Wv[:, cs]).astype(bf),
            "bq": np.ascontiguousarray((bq[cs] * scale).reshape(G, 1)),
            "bk": np.ascontiguousarray(bk[cs].reshape(G, 1)),
            "bv": np.ascontiguousarray(bv[cs]),
            "wo": np.ascontiguousarray(Wo[cs, :].reshape(NHL, 64, H)).astype(bf),
        })
    return in_maps


def kernel(x, Wq, bq, Wk, bk, Wv, bv, Wo, bo):
    from concourse.bass_utils import run_bass_kernel_spmd

    bo = np.asarray(bo, dtype=np.float32)
    nc = _build()
    in_maps = make_in_maps(x, Wq, bq, Wk, bk, Wv, bv, Wo)
    res = run_bass_kernel_spmd(nc, in_maps, core_ids=list(range(N_CORES)))

    out = np.empty((2, S, H), dtype=np.float32)
    for b in range(2):
        acc = res.results[4 * b]["out"].astype(np.float32)
        for g in range(1, 4):
            acc = acc + res.results[4 * b + g]["out"]
        out[b] = acc + bo
    return out
